# revision 5
# baseline (speedup 1.0000x reference)
"""Trainium2 Bass kernel for an attention block (B=8, T=2048, D=K=V=1024).

Reference math (per batch element, sharded one per NeuronCore):
    Q = x @ Wq.T + bq ; K = x @ Wk.T + bk ; V = x @ Wv.T + bv
    logits[t,s] = Q[t] . K[s],  masked -inf for s > t (strict upper tri)
    probs = softmax(logits, axis=t) / sqrt(1024)     # softmax over QUERY axis
    out = x + probs @ V

v5: all matmuls fp8 (e4m3) DoubleRow as in v4, but every PE transpose is
replaced by an xbar DMA transpose (dma_start(transpose=True), bf16,
[128,1024] -> [128,8,128] per instruction) so the PE only runs matmuls.
  - x tile chain: DMA f32 -> bf16 copy (kept as residual xb) -> xbar ->
    fp8 convert into xT8.  W tile chain: DMA f32 -> bf16 copy -> xbar ->
    fp8 convert with x32 scale into W*T8.
  - queue discipline: sync = bias DMAs + front x/Wq halves + all xbars +
    out DMAs; gpsimd = all other input DMAs (desc-gen only, no compute
    after the mask build so its queue never sem-blocks).
  - diagonal logits tiles are narrowed to the non-fully-masked columns
    (col >= 128*oi); the skipped Pq8 blocks that PV still reads via the
    DoubleRow round-up are memset to zero once at start.
  - residual: out = psum*(1/1024) + x via one vector scalar_tensor_tensor
    per half-tile (replaces the id1k PE matmuls of v4).
  - softmax over t as in v4: exp accumulates Z via accum_out; during the
    last t-block R=1/Z finalizes per s-tile; PV lags 2 steps.
Measured numerics: rel_err ~4.6e-3 (tolerance 2e-2).
"""

import time

import numpy as np

import concourse.bass as bass
import concourse.bacc as bacc
import concourse.mybir as mybir
import concourse.tile as tile
from concourse.bass_utils import run_bass_kernel_spmd

F32 = mybir.dt.float32
BF16 = mybir.dt.bfloat16
FP8 = mybir.dt.float8e4
AF = mybir.ActivationFunctionType
DR = mybir.MatmulPerfMode.DoubleRow
MULT = mybir.AluOpType.mult
ADD = mybir.AluOpType.add

P = 128          # partitions
T = 2048         # sequence length
D = 1024         # model dim
TB = 512         # t-block width
NTB = 4          # t-blocks
KO = 8           # k output tiles of 128
DK = 8           # contraction subtiles of 128
SV = 16          # s tiles of 128
NEG = -1.0e30
WS = 32.0        # weight quantization scale
PBASE = [0, 4, 12]   # flat index base of j<3 causal P tiles in Pbig


def _build_nc():
    nc = bacc.Bacc("TRN2", target_bir_lowering=False, debug=False, num_devices=8)

    x = nc.dram_tensor("x", [T, D], F32, kind="ExternalInput").ap()
    Wq = nc.dram_tensor("Wq", [D, D], F32, kind="ExternalInput").ap()
    bq = nc.dram_tensor("bq", [D], F32, kind="ExternalInput").ap()
    Wk = nc.dram_tensor("Wk", [D, D], F32, kind="ExternalInput").ap()
    bk = nc.dram_tensor("bk", [D], F32, kind="ExternalInput").ap()
    Wv = nc.dram_tensor("Wv", [D, D], F32, kind="ExternalInput").ap()
    bv = nc.dram_tensor("bv", [D], F32, kind="ExternalInput").ap()
    out = nc.dram_tensor("out", [T, D], F32, kind="ExternalOutput").ap()

    with tile.TileContext(nc) as tc:
        _kernel_body(nc, tc, x, Wq, bq, Wk, bk, Wv, bv, out)

    nc.compile()
    return nc


def _kernel_body(nc, tc, x, Wq, bq, Wk, bk, Wv, bv, out):
    from contextlib import ExitStack

    ctx = ExitStack()
    with ctx:
        consts = ctx.enter_context(tc.tile_pool(name="consts", bufs=1))
        wt8p = ctx.enter_context(tc.tile_pool(name="wt8", bufs=1))
        xt8p = ctx.enter_context(tc.tile_pool(name="xt8", bufs=1))
        kt8p = ctx.enter_context(tc.tile_pool(name="kt8", bufs=1))
        vp8p = ctx.enter_context(tc.tile_pool(name="vp8", bufs=1))
        pbigp = ctx.enter_context(tc.tile_pool(name="pbig", bufs=1))
        pq8p = ctx.enter_context(tc.tile_pool(name="pq8", bufs=1))
        qt8p = ctx.enter_context(tc.tile_pool(name="qt8", bufs=2))
        xbp = ctx.enter_context(tc.tile_pool(name="xb", bufs=1))
        pstp = ctx.enter_context(tc.tile_pool(name="pst", bufs=3))
        natp = ctx.enter_context(tc.tile_pool(name="nat", bufs=6))
        wbp = ctx.enter_context(tc.tile_pool(name="wb", bufs=3))
        xtp = ctx.enter_context(tc.tile_pool(name="xt", bufs=2))
        ostp = ctx.enter_context(tc.tile_pool(name="ost", bufs=3))
        psum_mm = ctx.enter_context(tc.tile_pool(name="psum_mm", bufs=8, space="PSUM"))

        # persistent fp8 operand tensors
        WqT8 = wt8p.tile([P, DK, D], FP8, name="WqT8")   # (32 Wq)^T [d_in, dk, k]
        WkT8 = wt8p.tile([P, DK, D], FP8, name="WkT8")
        WvT8 = wt8p.tile([P, DK, D], FP8, name="WvT8")
        xT8 = xt8p.tile([P, DK, T], FP8, name="xT8")     # x^T [d_in, dk, t]
        KT8 = kt8p.tile([P, KO, T], FP8, name="KT8")     # (K+bk)^T [k_in, ko, s]
        Vp8 = vp8p.tile([P, SV, D], FP8, name="Vp8")     # 32(V+bv) [s_in, sv, v]
        Pbig = pbigp.tile([P, 24, TB], BF16, name="Pbig")  # exp(logits), j<3
        Pq8 = [pq8p.tile([P, 4 * j + 4, TB], FP8, name=f"Pq8_{j}")
               for j in range(NTB)]                      # P/Z [s_in, sv, t] per j
        xb = xbp.tile([P, SV, D], BF16, name="xb")       # x rows bf16 (residual)

        # dead Pq8 blocks read by the PV DoubleRow round-up but never
        # written once the diagonal logits tiles are narrowed
        for j in range(NTB):
            nc.vector.memset(Pq8[j][:, 4 * j + 1, 0:P], 0.0)
            nc.vector.memset(Pq8[j][:, 4 * j + 3, 2 * P:3 * P], 0.0)

        Zacc = consts.tile([P, SV, NTB], F32, name="Zacc")
        nc.vector.memset(Zacc, 0.0)
        ztmp = consts.tile([P, SV], F32, name="ztmp")
        rtile = consts.tile([P, SV], F32, name="rtile")

        # biases first on sync (before any xbar joins that queue)
        bq_sb = consts.tile([P, KO], F32, name="bq_sb")
        nc.sync.dma_start(out=bq_sb, in_=bq.rearrange("(o p) -> p o", p=P))
        bk_sb = consts.tile([P, KO], F32, name="bk_sb")
        nc.sync.dma_start(out=bk_sb, in_=bk.rearrange("(o p) -> p o", p=P))

        # ---- front DMAs: x tiles 0-3 and Wq tiles 0-3 split gpsimd+sync ----
        def dma_in_split(dst, src, nsplit=2):
            step = P // nsplit
            for q in range(nsplit):
                eng = nc.gpsimd if q % 2 == 0 else nc.sync
                eng.dma_start(out=dst[q * step:(q + 1) * step, :],
                              in_=src[q * step:(q + 1) * step, :])

        xnat_pre = []
        for ti in range(4):
            xnat = natp.tile([P, D], F32, name="xnat", tag="nat")
            dma_in_split(xnat, x[ti * P:(ti + 1) * P, :], nsplit=2)
            xnat_pre.append(xnat)
        wq_pre = []
        for kt in range(4):
            wnat = natp.tile([P, D], F32, name="wnat", tag="nat")
            dma_in_split(wnat, Wq[kt * P:(kt + 1) * P, :], nsplit=2)
            wq_pre.append(wnat)

        # mask build on gpsimd compute (queued after the front desc-gen)
        mask_base = consts.tile([P, TB + 3 * P], BF16, name="mask_base")
        nc.gpsimd.memset(mask_base, 0.0)
        nc.gpsimd.affine_select(
            out=mask_base, in_=mask_base,
            compare_op=mybir.AluOpType.is_ge,
            fill=NEG,
            base=-(3 * P),
            pattern=[[1, TB + 3 * P]],
            channel_multiplier=-1,
        )
        masks = [mask_base[:, 3 * P - oi * P: 3 * P - oi * P + TB]
                 for oi in range(4)]

        # remaining input DMAs ride gpsimd only (desc-gen, never sem-blocks)
        def dma_gp(dst, src):
            nc.gpsimd.dma_start(out=dst, in_=src)

        eng_ctr = [0]

        def alt_copy(dst, src, scale=None):
            """Copy/scale-copy alternating between vector and scalar."""
            eng_ctr[0] += 1
            if eng_ctr[0] % 2 == 0:
                if scale is None:
                    nc.vector.tensor_copy(out=dst, in_=src)
                else:
                    nc.vector.tensor_scalar_mul(dst, src, scale)
            else:
                nc.scalar.activation(dst, src, AF.Copy,
                                     scale=1.0 if scale is None else scale)

        def emit_x_tile(ti, xnat=None):
            """x tile chain: (DMA'd) f32 -> xb bf16 -> xbar -> xT8 fp8."""
            t0 = ti * P
            if xnat is None:
                xnat = natp.tile([P, D], F32, name="xnat", tag="nat")
                dma_gp(xnat, x[t0:t0 + P, :])
            alt_copy(xb[:, ti, :], xnat)
            xt = xtp.tile([P, DK, P], BF16, name="xt", tag="xt")
            nc.sync.dma_start(out=xt, in_=xb[:, ti, :], transpose=True)
            alt_copy(xT8[:, :, t0:t0 + P], xt)

        def emit_w_tile(w_ap, dst, kt, wnat=None):
            """W row-tile chain: f32 -> bf16 -> xbar -> x32 fp8."""
            if wnat is None:
                wnat = natp.tile([P, D], F32, name="wnat", tag="nat")
                dma_gp(wnat, w_ap[kt * P:(kt + 1) * P, :])
            wb = wbp.tile([P, D], BF16, name="wb", tag="wb")
            alt_copy(wb, wnat)
            wt = xtp.tile([P, DK, P], BF16, name="wt", tag="xt")
            nc.sync.dma_start(out=wt, in_=wb, transpose=True)
            alt_copy(dst[:, :, kt * P:(kt + 1) * P], wt, scale=WS)

        def emit_x_block(j):
            for ts_ in range(TB // P):
                emit_x_tile(4 * j + ts_)

        def emit_qkt_ko(j, wt8, bias_sb, dst, ko):
            """One QT/KT column tile [k 128, t 512], fused epilogue -> fp8."""
            ps = psum_mm.tile([P, TB], F32, name="ps_qk", tag="mm")
            for a in range(4):
                nc.tensor.matmul(
                    ps,
                    lhsT=wt8[:, 2 * a:2 * a + 2, ko * P:(ko + 1) * P],
                    rhs=xT8[:, 2 * a:2 * a + 2, j * TB:(j + 1) * TB],
                    start=(a == 0), stop=(a == 3),
                    perf_mode=DR,
                )
            if ko % 2 == 0:
                nc.vector.tensor_scalar(
                    out=dst, in0=ps,
                    scalar1=1.0 / WS, scalar2=bias_sb[:, ko:ko + 1],
                    op0=MULT, op1=ADD,
                )
            else:
                nc.scalar.activation(
                    dst, ps, AF.Identity,
                    bias=bias_sb[:, ko:ko + 1], scale=1.0 / WS,
                )

        def emit_v_unit(j, si, h):
            """One Vp8 tile [s 128, v 512] = 32(V+bv) fp8."""
            sv = 4 * j + si
            s0 = sv * P
            ps = psum_mm.tile([P, TB], F32, name="ps_v", tag="mm")
            for a in range(4):
                nc.tensor.matmul(
                    ps,
                    lhsT=xT8[:, 2 * a:2 * a + 2, s0:s0 + P],
                    rhs=WvT8[:, 2 * a:2 * a + 2, h * TB:(h + 1) * TB],
                    start=(a == 0), stop=(a == 3),
                    perf_mode=DR,
                )
            nc.vector.tensor_add(
                out=Vp8[:, sv, h * TB:(h + 1) * TB],
                in0=ps, in1=bv32_sb[:, h * TB:(h + 1) * TB],
            )

        def emit_logits_exp(j, sv, qt8):
            """logits tile [s 128, t 512-col0] -> exp -> Pbig/pst; Z accum.

            Diagonal tiles (oi = sv-4j > 0) skip the fully-masked columns
            t < 128*oi."""
            oi = sv - 4 * j
            col0 = P * oi if oi > 0 else 0
            W = TB - col0
            ps = psum_mm.tile([P, TB], F32, name="ps_l", tag="mm")
            for a in range(4):
                nc.tensor.matmul(
                    ps[:, 0:W],
                    lhsT=KT8[:, 2 * a:2 * a + 2, sv * P:(sv + 1) * P],
                    rhs=qt8[:, 2 * a:2 * a + 2, col0:TB],
                    start=(a == 0), stop=(a == 3),
                    perf_mode=DR,
                )
            if oi >= 0:
                nc.vector.tensor_add(out=ps[:, 0:W], in0=ps[:, 0:W],
                                     in1=masks[oi][:, col0:TB])
            if j < NTB - 1:
                dst = Pbig[:, PBASE[j] + sv, col0:TB]
            else:
                pst = pstp.tile([P, TB], BF16, name="pst", tag="pst")
                dst = pst[:, 0:W]
            nc.scalar.activation(
                dst, ps[:, 0:W], AF.Exp, accum_out=Zacc[:, sv, j:j + 1],
            )
            return dst

        def emit_out_tile(i):
            """out rows [i*128, (i+1)*128): PV fp8 DR; fused epilogue
            out = psum/1024 + x on the vector engine."""
            jj = i // 4
            tc_ = i % 4
            npair = (i + 2) // 2
            for h in range(D // TB):
                ps = psum_mm.tile([P, TB], F32, name="ps_o", tag="mm")
                for a in range(npair):
                    nc.tensor.matmul(
                        ps,
                        lhsT=Pq8[jj][:, 2 * a:2 * a + 2, tc_ * P:(tc_ + 1) * P],
                        rhs=Vp8[:, 2 * a:2 * a + 2, h * TB:(h + 1) * TB],
                        start=(a == 0), stop=(a == npair - 1),
                        perf_mode=DR,
                    )
                oh = ostp.tile([P, TB], F32, name="oh", tag="ost")
                nc.vector.scalar_tensor_tensor(
                    out=oh, in0=ps, scalar=1.0 / (WS * WS),
                    in1=xb[:, i, h * TB:(h + 1) * TB],
                    op0=MULT, op1=ADD,
                )
                nc.sync.dma_start(
                    out=out[i * P:(i + 1) * P, h * TB:(h + 1) * TB], in_=oh)

        # ---- main pipeline ----
        for j in range(NTB):
            qt8 = qt8p.tile([P, KO, TB], FP8, name="qt8", tag="qt8")
            if j == 0:
                for ti in range(4):
                    emit_x_tile(ti, xnat=xnat_pre[ti])
                for kt in range(8):
                    emit_w_tile(Wq, WqT8, kt,
                                wnat=wq_pre[kt] if kt < 4 else None)
                    emit_qkt_ko(0, WqT8, bq_sb, qt8[:, kt, :], kt)
                for kt in range(8):
                    emit_w_tile(Wk, WkT8, kt)
                    emit_qkt_ko(0, WkT8, bk_sb, KT8[:, kt, 0:TB], kt)
                # bv broadcast + scale (needed by the first V epilogue)
                bv_sb = consts.tile([P, D], F32, name="bv_sb")
                bv_bcast = bass.AP(tensor=bv.tensor, offset=bv.offset,
                                   ap=[[0, P], [1, D]])
                nc.gpsimd.dma_start(out=bv_sb, in_=bv_bcast)
                bv32_sb = consts.tile([P, D], BF16, name="bv32_sb")
                nc.scalar.activation(bv32_sb, bv_sb, AF.Copy, scale=WS)
                for kt in range(8):
                    emit_w_tile(Wv, WvT8, kt)
                    if kt == 3:
                        for si in range(4):
                            emit_v_unit(0, si, 0)
                for si in range(4):
                    emit_v_unit(0, si, 1)
            else:
                for ko in range(KO):
                    emit_qkt_ko(j, WqT8, bq_sb, qt8[:, ko, :], ko)
                for ko in range(KO):
                    emit_qkt_ko(j, WkT8, bk_sb,
                                KT8[:, ko, j * TB:(j + 1) * TB], ko)
                for si in range(TB // P):
                    for h in range(D // TB):
                        emit_v_unit(j, si, h)

            if j < NTB - 1:
                emit_x_block(j + 1)

            for sv in range(4 * (j + 1)):
                pdst = emit_logits_exp(j, sv, qt8)
                if j == NTB - 1:
                    # Z[sv] final: R = 1/Z; normalize+convert column sv of
                    # every j' block to fp8; out-tiles lag 2 steps so the
                    # exp->R->convert chain stays off the PE critical path
                    nc.vector.reduce_sum(out=ztmp[:, sv:sv + 1],
                                         in_=Zacc[:, sv, :],
                                         axis=mybir.AxisListType.X)
                    nc.vector.reciprocal(rtile[:, sv:sv + 1],
                                         ztmp[:, sv:sv + 1])
                    for jp in range(NTB):
                        if sv > 4 * jp + 3:
                            continue
                        oi2 = sv - 4 * jp
                        col0 = P * oi2 if oi2 > 0 else 0
                        if jp == 3:
                            src = pdst  # already the [col0:TB] slice
                        else:
                            src = Pbig[:, PBASE[jp] + sv, col0:TB]
                        dstq = Pq8[jp][:, sv, col0:TB]
                        if (jp + sv) % 2 == 0:
                            nc.vector.tensor_scalar_mul(
                                dstq, src, rtile[:, sv:sv + 1])
                        else:
                            nc.scalar.activation(
                                dstq, src,
                                AF.Identity, scale=rtile[:, sv:sv + 1])
                    if sv >= 2:
                        emit_out_tile(sv - 2)
        emit_out_tile(SV - 2)
        emit_out_tile(SV - 1)


_NC_CACHE = None


def _get_nc():
    global _NC_CACHE
    if _NC_CACHE is None:
        _NC_CACHE = _build_nc()
    return _NC_CACHE


def kernel(minibatch, Wq, bq, Wk, bk, Wv, bv):
    minibatch = np.asarray(minibatch, dtype=np.float32)
    Wq = np.asarray(Wq, dtype=np.float32)
    bq = np.asarray(bq, dtype=np.float32)
    Wk = np.asarray(Wk, dtype=np.float32)
    bk = np.asarray(bk, dtype=np.float32)
    Wv = np.asarray(Wv, dtype=np.float32)
    bv = np.asarray(bv, dtype=np.float32)

    nc = _get_nc()
    B = minibatch.shape[0]
    in_maps = [
        {
            "x": np.ascontiguousarray(minibatch[i]),
            "Wq": Wq, "bq": bq, "Wk": Wk, "bk": bk, "Wv": Wv, "bv": bv,
        }
        for i in range(B)
    ]
    last_err = None
    for _attempt in range(3):
        try:
            res = run_bass_kernel_spmd(nc, in_maps, core_ids=list(range(B)))
            break
        except Exception as e:  # transient device errors
            last_err = e
            time.sleep(2.0)
    else:
        raise last_err
    return np.stack([res.results[i]["out"] for i in range(B)], axis=0)


# revision 7
# speedup vs baseline: 1.2123x; 1.2123x over previous
"""Trainium2 Bass kernel for an attention block (B=8, T=2048, D=K=V=1024).

Reference math (per batch element, sharded one per NeuronCore):
    Q = x @ Wq.T + bq ; K = x @ Wk.T + bk ; V = x @ Wv.T + bv
    logits[t,s] = Q[t] . K[s],  masked -inf for s > t (strict upper tri)
    probs = softmax(logits, axis=t) / sqrt(1024)     # softmax over QUERY axis
    out = x + probs @ V

v6: all matmuls fp8 (e4m3) DoubleRow; every transpose runs on the DMA
xbar (dma_start(transpose=True), bf16) in BATCHED form so the sync queue
only carries 16 transpose instructions total:
  - W row-group kt: Wq/Wk/Wv row-tiles are DMA'd f32, copied bf16 into
    one [128,3072] staging tile, transposed by ONE xbar into [128,24,128]
    and converted (x32) by ONE op into the fused WT8[w, dk, col] tensor.
  - x tiles transpose in pairs: [128,2048] -> [128,16,128] -> fp8 into a
    per-t-block xT8 tile (bufs=2; block j is only read during phase j).
  - queue discipline: sync = bias DMAs + front x halves + xbars + out
    DMAs; gpsimd = all other input DMAs (desc-gen only, never blocks).
  - diagonal logits tiles narrowed to columns >= 128*oi; the dead Pq8
    blocks that PV's DoubleRow round-up still reads are memset once.
  - residual epilogue: out = psum*(1/1024) + x via one vector
    scalar_tensor_tensor per half-tile (no id1k PE matmuls).
  - softmax over t: exp accumulates Z via accum_out; during the last
    t-block R=1/Z finalizes per s-tile; PV lags 2 steps.
Measured numerics: rel_err ~4.6e-3 (tolerance 2e-2).
"""

import time

import numpy as np

import concourse.bass as bass
import concourse.bacc as bacc
import concourse.mybir as mybir
import concourse.tile as tile
from concourse.bass_utils import run_bass_kernel_spmd

F32 = mybir.dt.float32
BF16 = mybir.dt.bfloat16
FP8 = mybir.dt.float8e4
AF = mybir.ActivationFunctionType
DR = mybir.MatmulPerfMode.DoubleRow
MULT = mybir.AluOpType.mult
ADD = mybir.AluOpType.add

P = 128          # partitions
T = 2048         # sequence length
D = 1024         # model dim
TB = 512         # t-block width
NTB = 4          # t-blocks
KO = 8           # k output tiles of 128
DK = 8           # contraction subtiles of 128
SV = 16          # s tiles of 128
NEG = -1.0e30
WS = 32.0        # weight quantization scale
PBASE = [0, 4, 12]   # flat index base of j<3 causal P tiles in Pbig


def _build_nc():
    nc = bacc.Bacc("TRN2", target_bir_lowering=False, debug=False, num_devices=8)

    x = nc.dram_tensor("x", [T, D], F32, kind="ExternalInput").ap()
    Wq = nc.dram_tensor("Wq", [D, D], F32, kind="ExternalInput").ap()
    bq = nc.dram_tensor("bq", [D], F32, kind="ExternalInput").ap()
    Wk = nc.dram_tensor("Wk", [D, D], F32, kind="ExternalInput").ap()
    bk = nc.dram_tensor("bk", [D], F32, kind="ExternalInput").ap()
    Wv = nc.dram_tensor("Wv", [D, D], F32, kind="ExternalInput").ap()
    bv = nc.dram_tensor("bv", [D], F32, kind="ExternalInput").ap()
    out = nc.dram_tensor("out", [T, D], F32, kind="ExternalOutput").ap()

    with tile.TileContext(nc) as tc:
        _kernel_body(nc, tc, x, Wq, bq, Wk, bk, Wv, bv, out)

    nc.compile()
    return nc


def _kernel_body(nc, tc, x, Wq, bq, Wk, bk, Wv, bv, out):
    from contextlib import ExitStack

    ctx = ExitStack()
    with ctx:
        consts = ctx.enter_context(tc.tile_pool(name="consts", bufs=1))
        wt8p = ctx.enter_context(tc.tile_pool(name="wt8", bufs=1))
        xt8p = ctx.enter_context(tc.tile_pool(name="xt8", bufs=2))
        kt8p = ctx.enter_context(tc.tile_pool(name="kt8", bufs=1))
        vp8p = ctx.enter_context(tc.tile_pool(name="vp8", bufs=1))
        pbigp = ctx.enter_context(tc.tile_pool(name="pbig", bufs=1))
        pq8p = ctx.enter_context(tc.tile_pool(name="pq8", bufs=1))
        qt8p = ctx.enter_context(tc.tile_pool(name="qt8", bufs=1))
        xbp = ctx.enter_context(tc.tile_pool(name="xb", bufs=1))
        pstp = ctx.enter_context(tc.tile_pool(name="pst", bufs=2))
        natp = ctx.enter_context(tc.tile_pool(name="nat", bufs=6))
        wbp = ctx.enter_context(tc.tile_pool(name="wb", bufs=2))
        xtp = ctx.enter_context(tc.tile_pool(name="xt", bufs=2))
        ostp = ctx.enter_context(tc.tile_pool(name="ost", bufs=2))
        psum_mm = ctx.enter_context(tc.tile_pool(name="psum_mm", bufs=8, space="PSUM"))

        # persistent fp8 operand tensors
        WT8 = wt8p.tile([P, 3, DK, D], FP8, name="WT8")  # (32 W)^T, w=q/k/v
        xT8b = [xt8p.tile([P, DK, TB], FP8, name="xT8", tag="xT8")
                for _ in range(NTB)]                     # x^T per t-block
        KT8 = kt8p.tile([P, KO, T], FP8, name="KT8")     # (K+bk)^T [k_in, ko, s]
        Vp8 = vp8p.tile([P, SV, D], FP8, name="Vp8")     # 32(V+bv) [s_in, sv, v]
        Pbig = pbigp.tile([P, 24, TB], BF16, name="Pbig")  # exp(logits), j<3
        Pq8 = [pq8p.tile([P, 4 * j + 4, TB], FP8, name=f"Pq8_{j}")
               for j in range(NTB)]                      # P/Z [s_in, sv, t] per j
        xb = xbp.tile([P, SV, D], BF16, name="xb")       # x rows bf16 (residual)

        # dead Pq8 blocks read by the PV DoubleRow round-up but never
        # written once the diagonal logits tiles are narrowed
        for j in range(NTB):
            nc.vector.memset(Pq8[j][:, 4 * j + 1, 0:P], 0.0)
            nc.vector.memset(Pq8[j][:, 4 * j + 3, 2 * P:3 * P], 0.0)

        Zacc = consts.tile([P, SV, NTB], F32, name="Zacc")
        nc.vector.memset(Zacc, 0.0)
        ztmp = consts.tile([P, SV], F32, name="ztmp")
        rtile = consts.tile([P, SV], F32, name="rtile")

        # biases first on sync (before any xbar joins that queue)
        bq_sb = consts.tile([P, KO], F32, name="bq_sb")
        nc.sync.dma_start(out=bq_sb, in_=bq.rearrange("(o p) -> p o", p=P))
        bk_sb = consts.tile([P, KO], F32, name="bk_sb")
        nc.sync.dma_start(out=bk_sb, in_=bk.rearrange("(o p) -> p o", p=P))

        # ---- front DMAs: x tiles 0-3 split gpsimd+sync; W kt=0,1 ----
        def dma_in_split(dst, src, nsplit=2):
            step = P // nsplit
            for q in range(nsplit):
                eng = nc.gpsimd if q % 2 == 0 else nc.sync
                eng.dma_start(out=dst[q * step:(q + 1) * step, :],
                              in_=src[q * step:(q + 1) * step, :])

        def dma_gp(dst, src):
            nc.gpsimd.dma_start(out=dst, in_=src)

        xnat_pre = []
        for ti in range(4):
            xnat = natp.tile([P, D], F32, name="xnat", tag="nat")
            dma_in_split(xnat, x[ti * P:(ti + 1) * P, :], nsplit=2)
            xnat_pre.append(xnat)

        W_APS = [Wq, Wk, Wv]
        wnat_q = []  # queue of DMA'd W row-tiles, 3 per kt in w order

        def dma_w_group(kt):
            for w in range(3):
                wnat = natp.tile([P, D], F32, name="wnat", tag="nat")
                dma_gp(wnat, W_APS[w][kt * P:(kt + 1) * P, :])
                wnat_q.append(wnat)

        dma_w_group(0)
        dma_w_group(1)

        # mask build on gpsimd compute (queued after the front desc-gen)
        mask_base = consts.tile([P, TB + 3 * P], BF16, name="mask_base")
        nc.gpsimd.memset(mask_base, 0.0)
        nc.gpsimd.affine_select(
            out=mask_base, in_=mask_base,
            compare_op=mybir.AluOpType.is_ge,
            fill=NEG,
            base=-(3 * P),
            pattern=[[1, TB + 3 * P]],
            channel_multiplier=-1,
        )
        masks = [mask_base[:, 3 * P - oi * P: 3 * P - oi * P + TB]
                 for oi in range(4)]

        # bv broadcast + scale (gpsimd DMA; needed by the first V epilogue)
        bv_nat = natp.tile([P, D], F32, name="bv_nat", tag="nat")
        bv_bcast = bass.AP(tensor=bv.tensor, offset=bv.offset,
                           ap=[[0, P], [1, D]])
        dma_gp(bv_nat, bv_bcast)
        bv32_sb = consts.tile([P, D], BF16, name="bv32_sb")
        nc.scalar.activation(bv32_sb, bv_nat, AF.Copy, scale=WS)

        eng_ctr = [0]

        def alt_copy(dst, src, scale=None):
            """Copy/scale-copy alternating between vector and scalar."""
            eng_ctr[0] += 1
            if eng_ctr[0] % 2 == 0:
                if scale is None:
                    nc.vector.tensor_copy(out=dst, in_=src)
                else:
                    nc.vector.tensor_scalar_mul(dst, src, scale)
            else:
                nc.scalar.activation(dst, src, AF.Copy,
                                     scale=1.0 if scale is None else scale)

        def emit_x_pair(ti, xnats=None):
            """x tiles ti, ti+1: f32 -> xb bf16 -> one xbar -> xT8 fp8."""
            j = ti // 4
            for u in range(2):
                if xnats is None:
                    xnat = natp.tile([P, D], F32, name="xnat", tag="nat")
                    dma_gp(xnat, x[(ti + u) * P:(ti + u + 1) * P, :])
                else:
                    xnat = xnats[u]
                alt_copy(xb[:, ti + u, :], xnat)
            xt = xtp.tile([P, 24, P], BF16, name="xt", tag="xt")
            nc.sync.dma_start(out=xt[:, 0:16, :], in_=xb[:, ti:ti + 2, :],
                              transpose=True)
            # xt[:, 8u+dk, :] = x-tile(ti+u) d-block dk transposed
            c0 = (ti % 4) * P
            for u in range(2):
                alt_copy(xT8b[j][:, :, c0 + u * P:c0 + (u + 1) * P],
                         xt[:, 8 * u:8 * u + 8, :])

        def emit_w_group(kt):
            """W row-group kt: 3 bf16 copies -> one xbar -> WT8 fp8 x32."""
            wb = wbp.tile([P, 3 * D], BF16, name="wb", tag="wb")
            for w in range(3):
                wnat = wnat_q.pop(0)
                alt_copy(wb[:, w * D:(w + 1) * D], wnat)
            wt = xtp.tile([P, 24, P], BF16, name="wt", tag="xt")
            nc.sync.dma_start(out=wt, in_=wb, transpose=True)
            # wt[:, 8w+dk, :] = W_w row-tile kt d-block dk transposed
            alt_copy(
                WT8[:, :, :, kt * P:(kt + 1) * P],
                wt.rearrange("p (w g) c -> p w g c", w=3),
                scale=WS)

        def emit_qkt_ko(j, w, bias_sb, dst, ko):
            """One QT/KT column tile [k 128, t 512], fused epilogue -> fp8."""
            ps = psum_mm.tile([P, TB], F32, name="ps_qk", tag="mm")
            for a in range(4):
                nc.tensor.matmul(
                    ps,
                    lhsT=WT8[:, w, 2 * a:2 * a + 2, ko * P:(ko + 1) * P],
                    rhs=xT8b[j][:, 2 * a:2 * a + 2, :],
                    start=(a == 0), stop=(a == 3),
                    perf_mode=DR,
                )
            if ko % 2 == 0:
                nc.vector.tensor_scalar(
                    out=dst, in0=ps,
                    scalar1=1.0 / WS, scalar2=bias_sb[:, ko:ko + 1],
                    op0=MULT, op1=ADD,
                )
            else:
                nc.scalar.activation(
                    dst, ps, AF.Identity,
                    bias=bias_sb[:, ko:ko + 1], scale=1.0 / WS,
                )

        def emit_v_unit(j, si, h):
            """One Vp8 tile [s 128, v 512] = 32(V+bv) fp8."""
            sv = 4 * j + si
            ps = psum_mm.tile([P, TB], F32, name="ps_v", tag="mm")
            for a in range(4):
                nc.tensor.matmul(
                    ps,
                    lhsT=xT8b[j][:, 2 * a:2 * a + 2, si * P:(si + 1) * P],
                    rhs=WT8[:, 2, 2 * a:2 * a + 2, h * TB:(h + 1) * TB],
                    start=(a == 0), stop=(a == 3),
                    perf_mode=DR,
                )
            nc.vector.tensor_add(
                out=Vp8[:, sv, h * TB:(h + 1) * TB],
                in0=ps, in1=bv32_sb[:, h * TB:(h + 1) * TB],
            )

        def emit_logits_exp(j, sv, qt8):
            """logits tile [s 128, t 512-col0] -> exp -> Pbig/pst; Z accum.

            Diagonal tiles (oi = sv-4j > 0) skip the fully-masked columns
            t < 128*oi."""
            oi = sv - 4 * j
            col0 = P * oi if oi > 0 else 0
            width = TB - col0
            ps = psum_mm.tile([P, TB], F32, name="ps_l", tag="mm")
            for a in range(4):
                nc.tensor.matmul(
                    ps[:, 0:width],
                    lhsT=KT8[:, 2 * a:2 * a + 2, sv * P:(sv + 1) * P],
                    rhs=qt8[:, 2 * a:2 * a + 2, col0:TB],
                    start=(a == 0), stop=(a == 3),
                    perf_mode=DR,
                )
            if oi >= 0:
                nc.vector.tensor_add(out=ps[:, 0:width], in0=ps[:, 0:width],
                                     in1=masks[oi][:, col0:TB])
            if j < NTB - 1:
                dst = Pbig[:, PBASE[j] + sv, col0:TB]
            else:
                pst = pstp.tile([P, TB], BF16, name="pst", tag="pst")
                dst = pst[:, 0:width]
            nc.scalar.activation(
                dst, ps[:, 0:width], AF.Exp, accum_out=Zacc[:, sv, j:j + 1],
            )
            return dst

        def emit_out_tile(i):
            """out rows [i*128, (i+1)*128): PV fp8 DR; fused epilogue
            out = psum/1024 + x on the vector engine."""
            jj = i // 4
            tc_ = i % 4
            npair = (i + 2) // 2
            for h in range(D // TB):
                ps = psum_mm.tile([P, TB], F32, name="ps_o", tag="mm")
                for a in range(npair):
                    nc.tensor.matmul(
                        ps,
                        lhsT=Pq8[jj][:, 2 * a:2 * a + 2, tc_ * P:(tc_ + 1) * P],
                        rhs=Vp8[:, 2 * a:2 * a + 2, h * TB:(h + 1) * TB],
                        start=(a == 0), stop=(a == npair - 1),
                        perf_mode=DR,
                    )
                oh = ostp.tile([P, TB], F32, name="oh", tag="ost")
                nc.vector.scalar_tensor_tensor(
                    out=oh, in0=ps, scalar=1.0 / (WS * WS),
                    in1=xb[:, i, h * TB:(h + 1) * TB],
                    op0=MULT, op1=ADD,
                )
                nc.sync.dma_start(
                    out=out[i * P:(i + 1) * P, h * TB:(h + 1) * TB], in_=oh)

        # ---- main pipeline ----
        for j in range(NTB):
            qt8 = qt8p.tile([P, KO, TB], FP8, name="qt8", tag="qt8")
            if j == 0:
                emit_x_pair(0, xnats=xnat_pre[0:2])
                emit_x_pair(2, xnats=xnat_pre[2:4])
                for kt in range(8):
                    if kt < 6:
                        dma_w_group(kt + 2)   # stay 2 groups ahead
                    emit_w_group(kt)
                    emit_qkt_ko(0, 0, bq_sb, qt8[:, kt, :], kt)
                    emit_qkt_ko(0, 1, bk_sb, KT8[:, kt, 0:TB], kt)
                    if kt == 3:
                        for si in range(4):
                            emit_v_unit(0, si, 0)
                for si in range(4):
                    emit_v_unit(0, si, 1)
            else:
                for ko in range(KO):
                    emit_qkt_ko(j, 0, bq_sb, qt8[:, ko, :], ko)
                for ko in range(KO):
                    emit_qkt_ko(j, 1, bk_sb,
                                KT8[:, ko, j * TB:(j + 1) * TB], ko)
                for si in range(TB // P):
                    for h in range(D // TB):
                        emit_v_unit(j, si, h)

            if j < NTB - 1:
                emit_x_pair(4 * (j + 1))
                emit_x_pair(4 * (j + 1) + 2)

            for sv in range(4 * (j + 1)):
                pdst = emit_logits_exp(j, sv, qt8)
                if j == NTB - 1:
                    # Z[sv] final: R = 1/Z; normalize+convert column sv of
                    # every j' block to fp8; out-tiles lag 2 steps so the
                    # exp->R->convert chain stays off the PE critical path
                    nc.vector.reduce_sum(out=ztmp[:, sv:sv + 1],
                                         in_=Zacc[:, sv, :],
                                         axis=mybir.AxisListType.X)
                    nc.vector.reciprocal(rtile[:, sv:sv + 1],
                                         ztmp[:, sv:sv + 1])
                    for jp in range(NTB):
                        if sv > 4 * jp + 3:
                            continue
                        oi2 = sv - 4 * jp
                        col0 = P * oi2 if oi2 > 0 else 0
                        if jp == 3:
                            src = pdst  # already the [col0:TB] slice
                        else:
                            src = Pbig[:, PBASE[jp] + sv, col0:TB]
                        dstq = Pq8[jp][:, sv, col0:TB]
                        if (jp + sv) % 2 == 0:
                            nc.vector.tensor_scalar_mul(
                                dstq, src, rtile[:, sv:sv + 1])
                        else:
                            nc.scalar.activation(
                                dstq, src,
                                AF.Identity, scale=rtile[:, sv:sv + 1])
                    if sv >= 2:
                        emit_out_tile(sv - 2)
        emit_out_tile(SV - 2)
        emit_out_tile(SV - 1)


_NC_CACHE = None


def _get_nc():
    global _NC_CACHE
    if _NC_CACHE is None:
        _NC_CACHE = _build_nc()
    return _NC_CACHE


def kernel(minibatch, Wq, bq, Wk, bk, Wv, bv):
    minibatch = np.asarray(minibatch, dtype=np.float32)
    Wq = np.asarray(Wq, dtype=np.float32)
    bq = np.asarray(bq, dtype=np.float32)
    Wk = np.asarray(Wk, dtype=np.float32)
    bk = np.asarray(bk, dtype=np.float32)
    Wv = np.asarray(Wv, dtype=np.float32)
    bv = np.asarray(bv, dtype=np.float32)

    nc = _get_nc()
    B = minibatch.shape[0]
    in_maps = [
        {
            "x": np.ascontiguousarray(minibatch[i]),
            "Wq": Wq, "bq": bq, "Wk": Wk, "bk": bk, "Wv": Wv, "bv": bv,
        }
        for i in range(B)
    ]
    last_err = None
    for _attempt in range(3):
        try:
            res = run_bass_kernel_spmd(nc, in_maps, core_ids=list(range(B)))
            break
        except Exception as e:  # transient device errors
            last_err = e
            time.sleep(2.0)
    else:
        raise last_err
    return np.stack([res.results[i]["out"] for i in range(B)], axis=0)


# revision 8
# speedup vs baseline: 1.5927x; 1.3138x over previous
"""Trainium2 Bass kernel for an attention block (B=8, T=2048, D=K=V=1024).

Reference math (per batch element, sharded one per NeuronCore):
    Q = x @ Wq.T + bq ; K = x @ Wk.T + bk ; V = x @ Wv.T + bv
    logits[t,s] = Q[t] . K[s],  masked -inf for s > t (strict upper tri)
    probs = softmax(logits, axis=t) / sqrt(1024)     # softmax over QUERY axis
    out = x + probs @ V

v7: all matmuls fp8 (e4m3) DoubleRow.  Two-region schedule:
  REGION 1 (projections): W row-tiles stream DMA -> x32 bf16 -> PE
    transpose (4 per PSUM tile, 1 drain) as in v4; x block 0 likewise on
    the PE, x blocks 1-3 via batched xbar DMA transposes ([128,2048] ->
    [128,16,128]) emitted a block ahead of use.  Q/K/V for each t-block
    run back-to-back per block; xT8 is transient (bufs=2) since block j
    is only read while projecting block j.  Q lands in 4 persistent
    k-major fp8 blocks, K in KT8, V in Vp8.
  REGION 2 (dense sweep, column-major over s): for each s-tile sv, the
    4 causal logits tiles (j = sv//4..3) -> exp (Z via accum_out) into
    transient tiles; Z -> R = 1/Z; normalize into Pq8 fp8; PV out-tile
    sv-2 lags so the exp->R chain stays off the PE critical path.  This
    removes all phase-boundary stalls and makes Pbig transient.
  - Diagonal logits tiles (j == sv//4) narrowed to columns >= 128*(sv%4);
    the dead Pq8 blocks that PV's DoubleRow round-up reads are memset.
  - Residual epilogue: out = psum*(1/1024) + x via one vector
    scalar_tensor_tensor per half-tile (no id1k matmuls).
Measured numerics: rel_err ~4.6e-3 (tolerance 2e-2).
"""

import time

import numpy as np

import concourse.bass as bass
import concourse.bacc as bacc
import concourse.mybir as mybir
import concourse.tile as tile
from concourse.bass_utils import run_bass_kernel_spmd
from concourse.masks import make_identity

F32 = mybir.dt.float32
BF16 = mybir.dt.bfloat16
FP8 = mybir.dt.float8e4
AF = mybir.ActivationFunctionType
DR = mybir.MatmulPerfMode.DoubleRow
MULT = mybir.AluOpType.mult
ADD = mybir.AluOpType.add

P = 128          # partitions
T = 2048         # sequence length
D = 1024         # model dim
TB = 512         # t-block width
NTB = 4          # t-blocks
KO = 8           # k output tiles of 128
DK = 8           # contraction subtiles of 128
SV = 16          # s tiles of 128
NEG = -1.0e30
WS = 32.0        # weight quantization scale


def _build_nc():
    nc = bacc.Bacc("TRN2", target_bir_lowering=False, debug=False, num_devices=8)

    x = nc.dram_tensor("x", [T, D], F32, kind="ExternalInput").ap()
    Wq = nc.dram_tensor("Wq", [D, D], F32, kind="ExternalInput").ap()
    bq = nc.dram_tensor("bq", [D], F32, kind="ExternalInput").ap()
    Wk = nc.dram_tensor("Wk", [D, D], F32, kind="ExternalInput").ap()
    bk = nc.dram_tensor("bk", [D], F32, kind="ExternalInput").ap()
    Wv = nc.dram_tensor("Wv", [D, D], F32, kind="ExternalInput").ap()
    bv = nc.dram_tensor("bv", [D], F32, kind="ExternalInput").ap()
    out = nc.dram_tensor("out", [T, D], F32, kind="ExternalOutput").ap()

    with tile.TileContext(nc) as tc:
        _kernel_body(nc, tc, x, Wq, bq, Wk, bk, Wv, bv, out)

    nc.compile()
    return nc


def _kernel_body(nc, tc, x, Wq, bq, Wk, bk, Wv, bv, out):
    from contextlib import ExitStack

    ctx = ExitStack()
    with ctx:
        consts = ctx.enter_context(tc.tile_pool(name="consts", bufs=1))
        wt8p = ctx.enter_context(tc.tile_pool(name="wt8", bufs=1))
        xt8p = ctx.enter_context(tc.tile_pool(name="xt8", bufs=2))
        kt8p = ctx.enter_context(tc.tile_pool(name="kt8", bufs=1))
        vp8p = ctx.enter_context(tc.tile_pool(name="vp8", bufs=1))
        pq8p = ctx.enter_context(tc.tile_pool(name="pq8", bufs=1))
        qt8p = ctx.enter_context(tc.tile_pool(name="qt8", bufs=1))
        xbp = ctx.enter_context(tc.tile_pool(name="xb", bufs=1))
        pexpp = ctx.enter_context(tc.tile_pool(name="pexp", bufs=6))
        natp = ctx.enter_context(tc.tile_pool(name="nat", bufs=7))
        wbp = ctx.enter_context(tc.tile_pool(name="wb", bufs=3))
        xtp = ctx.enter_context(tc.tile_pool(name="xt", bufs=2))
        ostp = ctx.enter_context(tc.tile_pool(name="ost", bufs=3))
        psum_t = ctx.enter_context(tc.tile_pool(name="psum_t", bufs=2, space="PSUM"))
        psum_mm = ctx.enter_context(tc.tile_pool(name="psum_mm", bufs=6, space="PSUM"))

        # identity gates the PE transposes at kernel start
        id32 = consts.tile([P, P], F32, name="id32")
        make_identity(nc, id32)
        idb = consts.tile([P, P], BF16, name="idb")
        nc.vector.tensor_copy(out=idb, in_=id32)

        # persistent fp8 operand tensors
        WqT8 = wt8p.tile([P, DK, D], FP8, name="WqT8")   # (32 Wq)^T [d_in, dk, k]
        WkT8 = wt8p.tile([P, DK, D], FP8, name="WkT8")
        WvT8 = wt8p.tile([P, DK, D], FP8, name="WvT8")
        xT8b = [xt8p.tile([P, DK, TB], FP8, name="xT8", tag="xT8")
                for _ in range(NTB)]                     # x^T per t-block
        KT8 = kt8p.tile([P, KO, T], FP8, name="KT8")     # (K+bk)^T [k_in, ko, s]
        Vp8 = vp8p.tile([P, SV, D], FP8, name="Vp8")     # 32(V+bv) [s_in, sv, v]
        QT8b = [qt8p.tile([P, KO, TB], FP8, name=f"QT8_{j}")
                for j in range(NTB)]                     # Q^T per t-block
        Pq8 = [pq8p.tile([P, 4 * j + 4, TB], FP8, name=f"Pq8_{j}")
               for j in range(NTB)]                      # P/Z [s_in, sv, t] per j
        xb = xbp.tile([P, SV, D], BF16, name="xb")       # x rows bf16 (residual)

        # dead Pq8 blocks read by the PV DoubleRow round-up but never
        # written once the diagonal logits tiles are narrowed
        for j in range(NTB):
            nc.vector.memset(Pq8[j][:, 4 * j + 1, 0:P], 0.0)
            nc.vector.memset(Pq8[j][:, 4 * j + 3, 2 * P:3 * P], 0.0)

        Zacc = consts.tile([P, SV, NTB], F32, name="Zacc")
        nc.vector.memset(Zacc, 0.0)
        ztmp = consts.tile([P, SV], F32, name="ztmp")
        rtile = consts.tile([P, SV], F32, name="rtile")

        # biases + bv broadcast early on sync (before any xbar)
        bq_sb = consts.tile([P, KO], F32, name="bq_sb")
        nc.sync.dma_start(out=bq_sb, in_=bq.rearrange("(o p) -> p o", p=P))
        bk_sb = consts.tile([P, KO], F32, name="bk_sb")
        nc.sync.dma_start(out=bk_sb, in_=bk.rearrange("(o p) -> p o", p=P))
        bv_sb = consts.tile([P, D], F32, name="bv_sb")
        bv_bcast = bass.AP(tensor=bv.tensor, offset=bv.offset,
                           ap=[[0, P], [1, D]])
        nc.sync.dma_start(out=bv_sb, in_=bv_bcast)

        # ---- front DMAs: x tiles 0-3 split gpsimd+sync; Wq 0-3 gpsimd ----
        def dma_in_split(dst, src, nsplit=2):
            step = P // nsplit
            for q in range(nsplit):
                eng = nc.gpsimd if q % 2 == 0 else nc.sync
                eng.dma_start(out=dst[q * step:(q + 1) * step, :],
                              in_=src[q * step:(q + 1) * step, :])

        def dma_gp(dst, src):
            nc.gpsimd.dma_start(out=dst, in_=src)

        xnat_pre = []
        for ti in range(4):
            xnat = natp.tile([P, D], F32, name="xnat", tag="nat")
            dma_in_split(xnat, x[ti * P:(ti + 1) * P, :], nsplit=2)
            xnat_pre.append(xnat)
        wq_pre = []
        for kt in range(4):
            wnat = natp.tile([P, D], F32, name="wnat", tag="nat")
            dma_gp(wnat, Wq[kt * P:(kt + 1) * P, :])
            wq_pre.append(wnat)

        # mask build on gpsimd compute
        mask_base = consts.tile([P, TB + 3 * P], BF16, name="mask_base")
        nc.gpsimd.memset(mask_base, 0.0)
        nc.gpsimd.affine_select(
            out=mask_base, in_=mask_base,
            compare_op=mybir.AluOpType.is_ge,
            fill=NEG,
            base=-(3 * P),
            pattern=[[1, TB + 3 * P]],
            channel_multiplier=-1,
        )
        masks = [mask_base[:, 3 * P - oi * P: 3 * P - oi * P + TB]
                 for oi in range(4)]

        bv32_sb = consts.tile([P, D], BF16, name="bv32_sb")
        nc.scalar.activation(bv32_sb, bv_sb, AF.Copy, scale=WS)

        eng_ctr = [0]

        def alt_copy(dst, src, scale=None):
            """Copy/scale-copy alternating between vector and scalar."""
            eng_ctr[0] += 1
            if eng_ctr[0] % 2 == 0:
                if scale is None:
                    nc.vector.tensor_copy(out=dst, in_=src)
                else:
                    nc.vector.tensor_scalar_mul(dst, src, scale)
            else:
                nc.scalar.activation(dst, src, AF.Copy,
                                     scale=1.0 if scale is None else scale)

        grp_ctr = [0]

        def transpose_group(srcb, dst, dk0, dst_col):
            """4 bf16 PE transposes into one [128,512] psum tile, 1 drain."""
            pt = psum_t.tile([P, 4 * P], BF16, name="pt", tag="pt")
            for q in range(4):
                dk = dk0 + q
                nc.tensor.transpose(
                    pt[:, q * P:(q + 1) * P],
                    srcb[:, dk * P:(dk + 1) * P], idb)
            dview = dst[:, dk0:dk0 + 4, dst_col:dst_col + P]
            pview = pt.rearrange("p (g c) -> p g c", g=4)
            if grp_ctr[0] % 2 == 0:
                nc.vector.tensor_copy(out=dview, in_=pview)
            else:
                nc.scalar.activation(dview, pview, AF.Copy)
            grp_ctr[0] += 1

        def emit_x_tile_pe(ti, xnat):
            """x tile via PE transpose (block 0 only: lowest latency)."""
            alt_copy(xb[:, ti, :], xnat)
            for dk0 in (0, 4):
                transpose_group(xb[:, ti, :], xT8b[0], dk0, (ti % 4) * P)

        def emit_x_pair_xbar(ti):
            """x tiles ti, ti+1: DMA f32 -> xb bf16 -> one xbar -> xT8 fp8."""
            j = ti // 4
            for u in range(2):
                xnat = natp.tile([P, D], F32, name="xnat", tag="nat")
                dma_gp(xnat, x[(ti + u) * P:(ti + u + 1) * P, :])
                alt_copy(xb[:, ti + u, :], xnat)
            xt = xtp.tile([P, 16, P], BF16, name="xt", tag="xt")
            nc.sync.dma_start(out=xt, in_=xb[:, ti:ti + 2, :], transpose=True)
            c0 = (ti % 4) * P
            for u in range(2):
                alt_copy(xT8b[j][:, :, c0 + u * P:c0 + (u + 1) * P],
                         xt[:, 8 * u:8 * u + 8, :])

        def emit_w_tile(w_ap, dst, kt, wnat=None):
            """W row-tile kt: (DMA'd) f32 -> x32 bf16 -> PE transpose."""
            if wnat is None:
                wnat = natp.tile([P, D], F32, name="wnat", tag="nat")
                dma_gp(wnat, w_ap[kt * P:(kt + 1) * P, :])
            wb = wbp.tile([P, D], BF16, name="wb", tag="wb")
            alt_copy(wb, wnat, scale=WS)
            for dk0 in (0, 4):
                transpose_group(wb, dst, dk0, kt * P)

        def emit_qkt_ko(j, wt8, bias_sb, dst, ko):
            """One QT/KT column tile [k 128, t 512], fused epilogue -> fp8."""
            ps = psum_mm.tile([P, TB], F32, name="ps_qk", tag="mm")
            for a in range(4):
                nc.tensor.matmul(
                    ps,
                    lhsT=wt8[:, 2 * a:2 * a + 2, ko * P:(ko + 1) * P],
                    rhs=xT8b[j][:, 2 * a:2 * a + 2, :],
                    start=(a == 0), stop=(a == 3),
                    perf_mode=DR,
                )
            if ko % 2 == 0:
                nc.vector.tensor_scalar(
                    out=dst, in0=ps,
                    scalar1=1.0 / WS, scalar2=bias_sb[:, ko:ko + 1],
                    op0=MULT, op1=ADD,
                )
            else:
                nc.scalar.activation(
                    dst, ps, AF.Identity,
                    bias=bias_sb[:, ko:ko + 1], scale=1.0 / WS,
                )

        def emit_v_unit(j, si, h):
            """One Vp8 tile [s 128, v 512] = 32(V+bv) fp8."""
            sv = 4 * j + si
            ps = psum_mm.tile([P, TB], F32, name="ps_v", tag="mm")
            for a in range(4):
                nc.tensor.matmul(
                    ps,
                    lhsT=xT8b[j][:, 2 * a:2 * a + 2, si * P:(si + 1) * P],
                    rhs=WvT8[:, 2 * a:2 * a + 2, h * TB:(h + 1) * TB],
                    start=(a == 0), stop=(a == 3),
                    perf_mode=DR,
                )
            nc.vector.tensor_add(
                out=Vp8[:, sv, h * TB:(h + 1) * TB],
                in0=ps, in1=bv32_sb[:, h * TB:(h + 1) * TB],
            )

        def emit_logits_exp(j, sv):
            """logits tile [s 128, t 512-col0] -> exp (Z accum) -> pexp.

            Only j == sv//4 can be diagonal; it skips the fully-masked
            columns t < 128*(sv%4)."""
            oi = sv - 4 * j
            col0 = P * oi if oi > 0 else 0
            width = TB - col0
            ps = psum_mm.tile([P, TB], F32, name="ps_l", tag="mm")
            for a in range(4):
                nc.tensor.matmul(
                    ps[:, 0:width],
                    lhsT=KT8[:, 2 * a:2 * a + 2, sv * P:(sv + 1) * P],
                    rhs=QT8b[j][:, 2 * a:2 * a + 2, col0:TB],
                    start=(a == 0), stop=(a == 3),
                    perf_mode=DR,
                )
            if oi >= 0:
                nc.vector.tensor_add(out=ps[:, 0:width], in0=ps[:, 0:width],
                                     in1=masks[oi][:, col0:TB])
            pexp = pexpp.tile([P, TB], BF16, name="pexp", tag="pexp")
            nc.scalar.activation(
                pexp[:, 0:width], ps[:, 0:width], AF.Exp,
                accum_out=Zacc[:, sv, j:j + 1],
            )
            return pexp

        def emit_out_tile(i):
            """out rows [i*128, (i+1)*128): PV fp8 DR; fused epilogue
            out = psum/1024 + x on the vector engine."""
            jj = i // 4
            tc_ = i % 4
            npair = (i + 2) // 2
            for h in range(D // TB):
                ps = psum_mm.tile([P, TB], F32, name="ps_o", tag="mm")
                for a in range(npair):
                    nc.tensor.matmul(
                        ps,
                        lhsT=Pq8[jj][:, 2 * a:2 * a + 2, tc_ * P:(tc_ + 1) * P],
                        rhs=Vp8[:, 2 * a:2 * a + 2, h * TB:(h + 1) * TB],
                        start=(a == 0), stop=(a == npair - 1),
                        perf_mode=DR,
                    )
                oh = ostp.tile([P, TB], F32, name="oh", tag="ost")
                nc.vector.scalar_tensor_tensor(
                    out=oh, in0=ps, scalar=1.0 / (WS * WS),
                    in1=xb[:, i, h * TB:(h + 1) * TB],
                    op0=MULT, op1=ADD,
                )
                nc.sync.dma_start(
                    out=out[i * P:(i + 1) * P, h * TB:(h + 1) * TB], in_=oh)

        # ---- REGION 1: projections ----
        # x block 0 on the PE, interleaved with early Wq DMAs
        for ti in range(4):
            emit_x_tile_pe(ti, xnat_pre[ti])
        for kt in range(8):
            if kt < 4:
                wnat = natp.tile([P, D], F32, name="wnat", tag="nat")
                dma_gp(wnat, Wq[(kt + 4) * P:(kt + 5) * P, :])
                wq_pre.append(wnat)
            emit_w_tile(Wq, WqT8, kt, wnat=wq_pre[kt])
            emit_qkt_ko(0, WqT8, bq_sb, QT8b[0][:, kt, :], kt)
        for kt in range(8):
            emit_w_tile(Wk, WkT8, kt)
            emit_qkt_ko(0, WkT8, bk_sb, KT8[:, kt, 0:TB], kt)
            if kt == 3:
                emit_x_pair_xbar(4)   # block 1
            if kt == 5:
                emit_x_pair_xbar(6)
        for kt in range(8):
            emit_w_tile(Wv, WvT8, kt)
            if kt == 3:
                for si in range(4):
                    emit_v_unit(0, si, 0)
        for si in range(4):
            emit_v_unit(0, si, 1)

        for j in range(1, NTB):
            if j < NTB - 1:
                emit_x_pair_xbar(4 * (j + 1))       # block j+1
                emit_x_pair_xbar(4 * (j + 1) + 2)
            for ko in range(KO):
                emit_qkt_ko(j, WqT8, bq_sb, QT8b[j][:, ko, :], ko)
            for ko in range(KO):
                emit_qkt_ko(j, WkT8, bk_sb,
                            KT8[:, ko, j * TB:(j + 1) * TB], ko)
            for si in range(TB // P):
                for h in range(D // TB):
                    emit_v_unit(j, si, h)

        # ---- REGION 2: dense column-major softmax/PV sweep ----
        for sv in range(SV):
            j0 = sv // 4
            pexps = {}
            for j in range(j0, NTB):
                pexps[j] = emit_logits_exp(j, sv)
            nc.vector.reduce_sum(out=ztmp[:, sv:sv + 1],
                                 in_=Zacc[:, sv, :],
                                 axis=mybir.AxisListType.X)
            nc.vector.reciprocal(rtile[:, sv:sv + 1], ztmp[:, sv:sv + 1])
            for jp in range(j0, NTB):
                oi2 = sv - 4 * jp
                col0 = P * oi2 if oi2 > 0 else 0
                src = pexps[jp][:, 0:TB - col0]
                dstq = Pq8[jp][:, sv, col0:TB]
                if (jp + sv) % 2 == 0:
                    nc.vector.tensor_scalar_mul(
                        dstq, src, rtile[:, sv:sv + 1])
                else:
                    nc.scalar.activation(
                        dstq, src, AF.Identity, scale=rtile[:, sv:sv + 1])
            if sv >= 2:
                emit_out_tile(sv - 2)
        emit_out_tile(SV - 2)
        emit_out_tile(SV - 1)


_NC_CACHE = None


def _get_nc():
    global _NC_CACHE
    if _NC_CACHE is None:
        _NC_CACHE = _build_nc()
    return _NC_CACHE


def kernel(minibatch, Wq, bq, Wk, bk, Wv, bv):
    minibatch = np.asarray(minibatch, dtype=np.float32)
    Wq = np.asarray(Wq, dtype=np.float32)
    bq = np.asarray(bq, dtype=np.float32)
    Wk = np.asarray(Wk, dtype=np.float32)
    bk = np.asarray(bk, dtype=np.float32)
    Wv = np.asarray(Wv, dtype=np.float32)
    bv = np.asarray(bv, dtype=np.float32)

    nc = _get_nc()
    B = minibatch.shape[0]
    in_maps = [
        {
            "x": np.ascontiguousarray(minibatch[i]),
            "Wq": Wq, "bq": bq, "Wk": Wk, "bk": bk, "Wv": Wv, "bv": bv,
        }
        for i in range(B)
    ]
    last_err = None
    for _attempt in range(3):
        try:
            res = run_bass_kernel_spmd(nc, in_maps, core_ids=list(range(B)))
            break
        except Exception as e:  # transient device errors
            last_err = e
            time.sleep(2.0)
    else:
        raise last_err
    return np.stack([res.results[i]["out"] for i in range(B)], axis=0)
